# revision 13
# baseline (speedup 1.0000x reference)
"""Trainium2 Bass kernel for nn_EncoderStack (dense transformer encoder layer).

Strategy (8 NeuronCores, single NEFF launch):
  Attention is head-parallel: each core owns 2 of 16 heads over all 4096
  tokens. scores^T = k q^T per head, softmax over the query axis (free
  dim), denominator folded into v. Per batch, an AllToAll exchanges o^T
  blocks so each core ends up with all 1024 attention features for its
  256-token slice of that batch. Wo + residual + norm + FFN + norm then
  run token-parallel.

  v2 changes vs the 554us baseline:
  - qkv projections run in fp8e4 with perf_mode=DoubleRow (256-deep
    contraction per pass). Wq/Wk/Wv are pre-scaled by 64 to clear the
    fp8 denormal cutoff; the descale folds into the exp scale (scores
    carry 64^2) and the vp z-normalization (v carries 64) at zero cost.
    The FFN stays bf16: fp8 there measured 1.3% output error per
    quantized tensor (4 tensors -> 2.7%, over the 2% gate), while fp8
    q/k/v costs only 6e-4 because attention averaging washes it out.
  - W1 is split per batch: batch 0's W1+relu is emitted as filler
    inside batch 1's ACT-bound attention window (per-batch N=256
    matmuls cost the same as merged N=512 in bf16), and W2(b0) plus its
    norm/output run during the last AllToAll's flight time, instead of
    the whole FFN serializing after attention.
  - W1/W2/wqkv stream into SBUF during attention so the FFN phase never
    waits on HBM.
  Normalization statistics stay fp32; scores/o-accumulation stay bf16
  (dk=64 cannot DoubleRow and exp on ScalarE paces attention anyway).
"""

import numpy as np

B, T, D = 2, 2048, 1024
H, DK, DV = 16, 64, 64
FF = 4096
N_CORES = 8
P = 128
TOK = B * T
TPB = T // N_CORES    # 256 tokens per core per batch
HPC = H // N_CORES    # 2 heads per core
KT = D // P           # 8
KTG = KT // 2         # 4 DoubleRow groups over D
FT = FF // P          # 32
FFG = FT // 2         # 16 DoubleRow groups over FF
ST = T // P           # 16
TT = TPB // P         # 2 token-tiles per core per batch
WS = 64.0             # fp8 weight pre-scale
IWS = 1.0 / WS
IWS2 = 1.0 / (WS * WS)

_CACHE = {}


def _build():
    import concourse.bacc as bacc
    import concourse.mybir as mybir
    from concourse import tile

    f32 = mybir.dt.float32
    bf16 = mybir.dt.bfloat16
    f8 = mybir.dt.float8e4
    AF = mybir.ActivationFunctionType
    ALU = mybir.AluOpType
    DR = mybir.MatmulPerfMode.DoubleRow

    nc = bacc.Bacc("TRN2", target_bir_lowering=False, debug=False,
                   enable_asserts=True, num_devices=N_CORES)

    xt_d = nc.dram_tensor("xt", [KTG, P, 2, TOK], f8, kind="ExternalInput")
    xres_d = nc.dram_tensor("xres", [2 * TPB, D], f32, kind="ExternalInput")
    wqkv_d = nc.dram_tensor("wqkv", [KTG, P, 2, 384], f8, kind="ExternalInput")
    wo_d = nc.dram_tensor("wo", [KT, P, D], bf16, kind="ExternalInput")
    w1_d = nc.dram_tensor("w1", [FT, KT, P, P], bf16, kind="ExternalInput")
    b1_d = nc.dram_tensor("b1", [P, FT], f32, kind="ExternalInput")
    w2_d = nc.dram_tensor("w2", [FT + 1, P, D], bf16, kind="ExternalInput")
    out_d = nc.dram_tensor("out", [2 * TPB, D], f32, kind="ExternalOutput")

    xres_r = xres_d.ap().rearrange("(a p) d -> a p d", p=P)
    out_r = out_d.ap().rearrange("(a p) d -> a p d", p=P)

    def drain(g):
        for _ in g:
            pass

    def zip2(main, filler, ratio=1):
        while True:
            try:
                next(main)
            except StopIteration:
                drain(filler)
                return
            for _ in range(ratio):
                try:
                    next(filler)
                except StopIteration:
                    drain(main)
                    return

    def chain(*gens):
        for g in gens:
            yield from g

    with tile.TileContext(nc) as tc:
        with tc.tile_pool(name="wts", bufs=1) as wts, \
             tc.tile_pool(name="small", bufs=6) as small, \
             tc.tile_pool(name="o1", bufs=2) as o1p, \
             tc.tile_pool(name="p1", bufs=2) as p1, \
             tc.tile_pool(name="p2", bufs=2) as p2, \
             tc.tile_pool(name="ps", bufs=2, space="PSUM") as psp, \
             tc.tile_pool(name="dram", bufs=1, space="DRAM") as dram:

            wqkv_sb = wts.tile([P, KTG * 2 * 384], f8)
            for g in range(KTG):
                nc.sync.dma_start(
                    wqkv_sb[:, g * 768:(g + 1) * 768]
                        .rearrange("p (j m) -> p j m", j=2),
                    wqkv_d.ap()[g])
            wqkv_r = wqkv_sb[:].rearrange("p (g j m) -> p g j m", g=KTG, j=2)

            wo_sb = wts.tile([P, KT * D], bf16)
            nc.sync.dma_start(
                wo_sb[:].rearrange("p (a m) -> p a m", a=KT),
                wo_d.ap().rearrange("a p m -> p a m"))
            b1_sb = wts.tile([P, FT], f32)
            nc.sync.dma_start(b1_sb[:], b1_d.ap())
            ones_sb = wts.tile([P, P], bf16)
            nc.vector.memset(ones_sb[:], 0.0)
            nc.vector.memset(ones_sb[0:1, :], 1.0)

            a2a_in = [[dram.tile([N_CORES, 64, TPB], bf16, tag=f"ain{b}{h}",
                               name=f"ain{b}{h}") for h in range(HPC)]
                      for b in range(B)]
            a2a_out = [[dram.tile([N_CORES, 64, TPB], bf16, tag=f"aout{b}{h}",
                                name=f"aout{b}{h}") for h in range(HPC)]
                       for b in range(B)]

            def emit_a2a(b, h):
                nc.gpsimd.collective_compute(
                    "AllToAll", ALU.bypass,
                    replica_groups=[list(range(N_CORES))],
                    ins=[a2a_in[b][h].opt()], outs=[a2a_out[b][h].opt()])

            out1T_all = o1p.tile([P, KT * 2 * TPB], bf16, tag="out1t",
                                 bufs=1, name="out1T_all")
            h1T_all = p2.tile([P, FT * 2 * TPB], bf16, tag="h1t", bufs=1,
                              name="h1T_all")
            h1T_r = h1T_all[:].rearrange("p (f x) -> p f x", f=FT)
            q_sb = [None, None]
            k_sb = [None, None]
            v_sb = [None, None]
            oall_sb = [None, None]
            out1b_sb = [None, None]

            def gen_qkv(b):
                xt_b = p1.tile([P, KTG * 2 * T], f8, tag="xt", bufs=1, name="xt_b")
                for g in range(KTG):
                    nc.sync.dma_start(
                        xt_b[:, g * 2 * T:(g + 1) * 2 * T]
                            .rearrange("p (j t) -> p j t", j=2),
                        xt_d.ap()[g][:, :, b * T:(b + 1) * T])
                xt_r = xt_b[:].rearrange("p (g j t) -> p g j t", g=KTG, j=2)
                yield
                q_sb[b] = p1.tile([P, T], bf16, tag="q", name="q_sb")
                k_sb[b] = p1.tile([P, T], bf16, tag="k", name="k_sb")
                v_sb[b] = p1.tile([P, T], bf16, tag="v", name="v_sb")
                for dst, wofs in ((q_sb[b], 0), (k_sb[b], P)):
                    for half in range(2):
                        pt = psp.tile([P, 1024], f32, tag="mm", name="pt")
                        for c in range(2):
                            ofs = half * 1024 + c * 512
                            for g in range(KTG):
                                nc.tensor.matmul(
                                    pt[:, c * 512:(c + 1) * 512],
                                    wqkv_r[:, g, :, wofs:wofs + P],
                                    xt_r[:, g, :, ofs:ofs + 512],
                                    perf_mode=DR,
                                    start=(g == 0), stop=(g == KTG - 1))
                            yield
                        nc.vector.tensor_copy(
                            dst[:, half * 1024: half * 1024 + 1024], pt[:])
                        yield
                for st in range(ST):
                    pv = psp.tile([P, P], f32, tag="mm", name="pv")
                    for kt in range(KT):
                        g, j = kt // 2, kt % 2
                        nc.tensor.matmul(
                            pv[:],
                            xt_r[:, g, j, st * P:(st + 1) * P],
                            wqkv_r[:, g, j, 256:384],
                            start=(kt == 0), stop=(kt == KT - 1))
                    nc.vector.tensor_copy(v_sb[b][:, st * P:(st + 1) * P], pv[:])
                    yield

            EXPSC = 0.125 * IWS2   # 1/sqrt(dk) folded with the q,k fp8 scales

            def gen_attn(b):
                for h in range(HPC):
                    hofs = 64 * h
                    o_ps = psp.tile([64, T], f32, tag="o", bufs=1, name="o_ps")
                    for st in range(ST):
                        at_tiles = [None, None]
                        zp = small.tile([P, 2], f32, tag="zp", name="zp")
                        for half in range(2):
                            sc = psp.tile([P, 1024], f32, tag="mm", name="sc")
                            for c in range(2):
                                ofs = half * 1024 + c * 512
                                nc.tensor.matmul(
                                    sc[:, c * 512:(c + 1) * 512],
                                    k_sb[b][hofs:hofs + 64, st * P:(st + 1) * P],
                                    q_sb[b][hofs:hofs + 64, ofs: ofs + 512],
                                    start=True, stop=True)
                            at = p1.tile([P, 1024], bf16, tag="at", bufs=6, name="at")
                            nc.scalar.activation(at[:], sc[:], AF.Exp, scale=EXPSC,
                                                 accum_out=zp[:, half:half + 1])
                            at_tiles[half] = at
                        yield
                        zs = small.tile([P, 1], f32, tag="zs", name="zs")
                        nc.vector.tensor_add(zs[:], zp[:, 0:1], zp[:, 1:2])
                        zi = small.tile([P, 1], f32, tag="zi", name="zi")
                        nc.vector.reciprocal(zi[:], zs[:])
                        vp = small.tile([P, 64], bf16, tag="vp", name="vp")
                        nc.vector.tensor_scalar(
                            vp[:], v_sb[b][:, st * P + hofs: st * P + hofs + 64],
                            zi[:], IWS, op0=ALU.mult, op1=ALU.mult)
                        for half in range(2):
                            for c in range(2):
                                ofs = half * 1024 + c * 512
                                nc.tensor.matmul(
                                    o_ps[:, ofs: ofs + 512],
                                    vp[:],
                                    at_tiles[half][:, c * 512:(c + 1) * 512],
                                    start=(st == 0), stop=(st == ST - 1))
                        yield
                    ot = p1.tile([64, T], bf16, tag="ot", bufs=2, name="ot")
                    nc.vector.tensor_copy(ot[:], o_ps[:])
                    for j in range(N_CORES):
                        nc.sync.dma_start(a2a_in[b][h][j],
                                          ot[:, j * TPB:(j + 1) * TPB])
                    emit_a2a(b, h)

            i32 = mybir.dt.int32
            magic1 = wts.tile([P, 1], i32)
            nc.vector.memset(magic1[:], 0x5f3759e0)

            def norm_rows(y_ap, ssum, out_ap):
                negmean = small.tile([P, 1], f32, tag="st2", name="negmean")
                nc.scalar.mul(negmean[:], ssum[:], -1.0 / D)
                sq = p2.tile([P, D], bf16, tag="sq", bufs=1, name="sq")
                ssq = small.tile([P, 1], f32, tag="st4", name="ssq")
                nc.scalar.activation(sq[:], y_ap, AF.Square,
                                     bias=negmean[:], accum_out=ssq[:])
                # istd = rsqrt(ssq/(D-1)) via magic-constant seed + 2 Newton
                # steps, all tiny [P,1] DVE ops (keeps ScalarE in one
                # activation table set - no ACT_TABLE_LOAD churn).
                v = small.tile([P, 1], f32, tag="st5", name="v")
                nc.vector.tensor_scalar_mul(v[:], ssq[:], 1.0 / (D - 1))
                yh = small.tile([P, 1], i32, tag="st6", name="yh")
                nc.vector.tensor_scalar(yh[:], v[:].bitcast(i32), 1, None,
                                        op0=ALU.logical_shift_right)
                yn = small.tile([P, 1], i32, tag="st12", name="yn")
                nc.vector.tensor_scalar(yn[:], yh[:], -1, None,
                                        op0=ALU.bitwise_xor)
                y0 = small.tile([P, 1], i32, tag="st7", name="y0")
                nc.vector.tensor_add(y0[:], yn[:], magic1[:])
                istd = y0[:].bitcast(f32)
                for _ in range(2):
                    aa = small.tile([P, 1], f32, tag="st8", name="aa")
                    nc.vector.tensor_mul(aa[:], istd, istd)
                    bb = small.tile([P, 1], f32, tag="st9", name="bb")
                    nc.vector.tensor_mul(bb[:], v[:], aa[:])
                    cc = small.tile([P, 1], f32, tag="st10", name="cc")
                    nc.vector.tensor_scalar(cc[:], bb[:], -0.5, 1.5,
                                            op0=ALU.mult, op1=ALU.add)
                    ny = small.tile([P, 1], f32, tag="st11", name="ny")
                    nc.vector.tensor_mul(ny[:], cc[:], istd)
                    istd = ny[:]
                nc.vector.tensor_scalar(out_ap, y_ap, negmean[:], istd,
                                        op0=ALU.add, op1=ALU.mult)

            def gen_p2a(b):
                # gather attention features for my tokens of batch b
                oall_sb[b] = o1p.tile([P, KT * TPB], bf16, tag="oall", name="oall_sb")
                for kt in range(KT):
                    for h in range(HPC):
                        nc.sync.dma_start(
                            oall_sb[b][64 * h:64 * h + 64, kt * TPB:(kt + 1) * TPB],
                            a2a_out[b][h][kt])
                out1b_sb[b] = o1p.tile([P, TT * D], bf16, tag="out1b", name="out1b_sb")
                for tt in range(TT):
                    pw = psp.tile([P, D], f32, tag="mm", name="pw")
                    for kt in range(KT):
                        for c in range(2):
                            nc.tensor.matmul(
                                pw[:, c * 512:(c + 1) * 512],
                                oall_sb[b][:, kt * TPB + tt * P: kt * TPB + (tt + 1) * P],
                                wo_sb[:, kt * D + c * 512: kt * D + (c + 1) * 512],
                                start=(kt == 0), stop=(kt == KT - 1))
                        if kt % 4 == 3:
                            yield
                    xr = p2.tile([P, D], f32, tag="xr", name="xr")
                    nc.sync.dma_start(xr[:], xres_r[b * TT + tt])
                    y = p2.tile([P, D], f32, tag="y", name="y")
                    ssum = small.tile([P, 1], f32, tag="st1", name="ssum")
                    nc.vector.scalar_tensor_tensor(
                        y[:], pw[:], 0.0, xr[:], op0=ALU.add, op1=ALU.add,
                        accum_out=ssum[:])
                    norm_rows(y[:], ssum, out1b_sb[b][:, tt * D:(tt + 1) * D])
                    yield
                    for kt in range(KT):
                        nc.sync.dma_start_transpose(
                            out1T_all[:, kt * 2 * TPB + b * TPB + tt * P:
                                      kt * 2 * TPB + b * TPB + (tt + 1) * P],
                            out1b_sb[b][:, tt * D + kt * P: tt * D + (kt + 1) * P])
                    yield

            def gen_w1(b):
                for ft in range(FT):
                    w1s = p2.tile([P, KT * P], bf16, tag="w1s", bufs=4, name="w1s")
                    nc.sync.dma_start(
                        w1s[:].rearrange("p (a m) -> p a m", a=KT),
                        w1_d.ap()[ft].rearrange("a p m -> p a m"))
                    ph = psp.tile([P, TPB], f32, tag="mm", name="ph")
                    for kt in range(KT):
                        nc.tensor.matmul(
                            ph[:],
                            w1s[:, kt * P:(kt + 1) * P],
                            out1T_all[:, kt * 2 * TPB + b * TPB:
                                      kt * 2 * TPB + (b + 1) * TPB],
                            start=(kt == 0), stop=(kt == KT - 1))
                    nc.vector.tensor_scalar(
                        h1T_r[:, ft, b * TPB:(b + 1) * TPB], ph[:],
                        b1_sb[:, ft:ft + 1], 0.0, op0=ALU.add, op1=ALU.max)
                    yield

            def gen_w2norm(b):
                pfs = [psp.tile([P, D], f32, tag="mm", name=f"pf{tt}")
                       for tt in range(TT)]
                for ft in range(FT + 1):
                    w2s = p2.tile([P, D], bf16, tag="w2s", bufs=8, name="w2s")
                    nc.sync.dma_start(w2s[:], w2_d.ap()[ft])
                    last = ft == FT
                    for tt in range(TT):
                        lhsT = (ones_sb[:] if last else
                                h1T_r[:, ft, b * TPB + tt * P:
                                      b * TPB + (tt + 1) * P])
                        for c in range(2):
                            nc.tensor.matmul(
                                pfs[tt][:, c * 512:(c + 1) * 512],
                                lhsT,
                                w2s[:, c * 512:(c + 1) * 512],
                                start=(ft == 0), stop=last)
                    yield
                for tt in range(TT):
                    y2 = p2.tile([P, D], f32, tag="y", name="y2")
                    ssum = small.tile([P, 1], f32, tag="st1", name="ssum2")
                    nc.vector.scalar_tensor_tensor(
                        y2[:], pfs[tt][:], 0.0,
                        out1b_sb[b][:, tt * D:(tt + 1) * D],
                        op0=ALU.add, op1=ALU.add, accum_out=ssum[:])
                    o2 = p2.tile([P, D], f32, tag="o2", name="o2")
                    norm_rows(y2[:], ssum, o2[:])
                    nc.sync.dma_start(out_r[b * TT + tt], o2[:])
                    yield

            # ---------------- emission schedule ----------------
            drain(gen_qkv(0))
            attn0 = gen_attn(0)
            qkv1 = gen_qkv(1)
            next(qkv1)          # emit xt(b1) load early
            for _ in range(8):  # solo prefix while xt(b1) streams in
                next(attn0)
            zip2(attn0, qkv1, ratio=1)
            attn1 = gen_attn(1)
            fill1 = chain(gen_p2a(0), gen_w1(0))
            for _ in range(12):   # solo prefix: let the A2A-0 halves land
                next(attn1)
            zip2(attn1, fill1, ratio=1)
            # batch-0 FFN tail runs while the last AllToAll is in flight
            drain(gen_w2norm(0))
            drain(gen_p2a(1))
            drain(gen_w1(1))
            drain(gen_w2norm(1))

    nc.compile()
    return nc


def _get_nc():
    if "nc" not in _CACHE:
        _CACHE["nc"] = _build()
    return _CACHE["nc"]


def _prep_inputs(x, Wq, Wk, Wv, Wo, W1, b1, W2, b2):
    import ml_dtypes
    bf = ml_dtypes.bfloat16
    f8 = ml_dtypes.float8_e4m3
    x = np.asarray(x, np.float32)
    x2 = np.ascontiguousarray(x.reshape(TOK, D))
    # x^T packed for DoubleRow: (g, r, j, tok) = x2[tok, 256g + 128j + r]
    xt = np.ascontiguousarray(
        x2.T.reshape(KTG, 2, P, TOK).transpose(0, 2, 1, 3)).astype(f8)
    wo8 = np.ascontiguousarray(np.asarray(Wo, np.float32).astype(bf).reshape(KT, P, D))
    w1t = np.ascontiguousarray(
        np.asarray(W1, np.float32).astype(bf).reshape(KT, P, FT, P)
        .transpose(2, 0, 1, 3))
    b2blk = np.zeros((1, P, D), np.float32)
    b2blk[0, 0, :] = np.asarray(b2, np.float32)
    w2t = np.ascontiguousarray(np.concatenate(
        [np.asarray(W2, np.float32).reshape(FT, P, D), b2blk], axis=0).astype(bf))
    b1t = np.ascontiguousarray(np.asarray(b1, np.float32).reshape(FT, P).T)
    Wq = np.asarray(Wq, np.float32) * WS
    Wk = np.asarray(Wk, np.float32) * WS
    Wv = np.asarray(Wv, np.float32) * WS
    in_maps = []
    for c in range(N_CORES):
        h0 = HPC * c
        wqkv = np.concatenate(
            [Wq[h0], Wq[h0 + 1], Wk[h0], Wk[h0 + 1], Wv[h0], Wv[h0 + 1]],
            axis=1)  # [D, 384]
        wqkv = np.ascontiguousarray(
            wqkv.reshape(KTG, 2, P, 384).transpose(0, 2, 1, 3)).astype(f8)
        xres = np.ascontiguousarray(np.concatenate(
            [x2[c * TPB:(c + 1) * TPB],
             x2[T + c * TPB: T + (c + 1) * TPB]], axis=0))
        in_maps.append({
            "xt": xt, "xres": xres, "wqkv": wqkv, "wo": wo8,
            "w1": w1t, "b1": b1t, "w2": w2t,
        })
    return in_maps


def _assemble(results):
    out = np.empty((TOK, D), np.float32)
    for c in range(N_CORES):
        r = np.asarray(results[c]["out"], np.float32)
        out[c * TPB:(c + 1) * TPB] = r[:TPB]
        out[T + c * TPB: T + (c + 1) * TPB] = r[TPB:]
    return out.reshape(B, T, D)


def kernel(x, Wq, Wk, Wv, Wo, W1, b1, W2, b2):
    from concourse.bass_utils import run_bass_kernel_spmd
    nc = _get_nc()
    in_maps = _prep_inputs(x, Wq, Wk, Wv, Wo, W1, b1, W2, b2)
    res = run_bass_kernel_spmd(nc, in_maps, core_ids=list(range(N_CORES)))
    return _assemble(res.results)
